# revision 46
# baseline (speedup 1.0000x reference)
"""nn_MultiHeadAttention kernel for 8 Trainium2 NeuronCores.

Sharding: 8 cores = 4 batches (data parallel) x 2 head-groups of 8 heads
(tensor parallel). Each core computes its batch's QKV projection for its
head group (column-parallel), RoPE, causal attention, and a partial
out-projection (row-parallel). Host sums the two partials per batch and
adds the output bias.

Design (vs the DRAM-spill f32r baseline):
  - All matmul operands bf16 (host pre-converts); fp32 PSUM accumulation.
  - Everything SBUF-resident: x (8MB), q/k rows (8MB, roped in place),
    v (4MB), attn (4MB). No DRAM scratch roundtrips.
  - Weights host-prearranged to [128, ...] layouts so every weight DMA is
    a contiguous slice (4KB/partition lines).
  - Phase order: V-proj (first Q tile interleaved to hide the wv chunk
    reload), Q/K-proj (head-major), per-head attention, out-proj (first
    weight chunk prefetched during attention). RoPE runs in place on the
    q/k tiles via a partition-swap SBUF-SBUF DMA + 3 DVE ops, emitted
    mid-stream so it never blocks the DVE FIFO at a head start.
  - Attention processes two heads' chunks interleaved step-by-step so the
    PE always has independent work while ScalarE runs exp, with score
    matmuls one j-pair ahead of the PV matmuls. Causal structure:
    j-tiles in descending order with diagonal tiles width-narrowed; only
    the 128x128 diagonal block is masked, by accumulating a shared
    triangular table through a 128-row identity matmul.
  - Scores stay transposed S^T[k,q]; exp on ScalarE straight from PSUM
    with the 1/sqrt(D) scale fused. The softmax denominator accumulates
    on the DVE (SBUF bf16 tile, one add per pexp tile) with a single
    ones-column matmul per chunk — keeping PE work per pipeline cycle
    well below the ScalarE exp pace so HAM clock-gate oscillation cannot
    throttle the PE (this alone was worth ~73us/iter).
  - Normalization is decoupled from the PSUM critical path: unnormalized
    P@V evicts immediately (DVE); the 1/Z multiply (DVE reciprocal +
    GPSIMD partition-broadcast) is deferred one chunk.
"""

import os
import sys

if "/opt/trn_rl_repo" not in sys.path:
    sys.path.insert(0, "/opt/trn_rl_repo")

ROPE_DMA = os.environ.get("K_ROPE_DMA", "gpsimd")   # sync | gpsimd
X_SPLIT = int(os.environ.get("K_X_SPLIT", "2"))     # number of x DMAs

import numpy as np
import ml_dtypes

import concourse.bass as bass
import concourse.bacc as bacc
import concourse.mybir as mybir
import concourse.tile as tile
from concourse.bass_utils import run_bass_kernel_spmd

F32 = mybir.dt.float32
BF16 = mybir.dt.bfloat16
BF_NP = ml_dtypes.bfloat16

B, T, C = 4, 2048, 2048
H = 16            # total heads
HG = 8            # heads per core (group)
D = 128           # head dim
GC = HG * D       # channels per group = 1024
SCALE = 1.0 / float(np.sqrt(D))
MASKVAL = -30000.0
N_CORES = 8

KT = C // 128     # 16 contraction tiles
TT = T // 128     # 16 T tiles
TC = T // 512     # 4 T chunks of 512
ND = 2            # v output chunks of 512


def build_program(iters=1, unroll=1, phases=(1, 2, 3)):
    nc = bacc.Bacc("TRN2", target_bir_lowering=False, debug=False)

    # x pre-tiled: [p=128, k(16), t(2048)] — block k holds x^T rows k*128..
    x2 = nc.dram_tensor("x2", [128, KT * T], BF16, kind="ExternalInput").ap()
    # j-major (j = q0,k0,q1,k1,...): [p=128, j(16), k(16), c(128)]
    wqk = nc.dram_tensor("wqk", [128, 16 * KT * 128], BF16,
                         kind="ExternalInput").ap()
    # [p=128 (out dim within block), j(16)]
    bqk = nc.dram_tensor("bqk", [128, 16], F32, kind="ExternalInput").ap()
    # nd-major: [p=128, nd(2), k(16), c(512)]
    wv = nc.dram_tensor("wv", [128, ND * KT * 512], BF16,
                        kind="ExternalInput").ap()
    # csub-major: [p=128 (d), csub(16), h(8), c(128)]
    wo = nc.dram_tensor("wo", [128, 16 * HG * 128], BF16,
                        kind="ExternalInput").ap()
    # packed consts: sincos = [sin2 | cos2], ctri = [tri | ident | onescol],
    # crow = [ones128 | bv]
    sincos = nc.dram_tensor("sincos", [128, 2 * T], BF16,
                            kind="ExternalInput").ap()
    ctri = nc.dram_tensor("ctri", [128, 257], BF16,
                          kind="ExternalInput").ap()
    crow = nc.dram_tensor("crow", [1, 128 + GC], BF16,
                          kind="ExternalInput").ap()
    # y^T stored [p(128), csub(16), t(2048)]; host reassembles
    y = nc.dram_tensor("y", [128, 16, T], BF16,
                       kind="ExternalOutput").ap()

    with tile.TileContext(nc) as tc:
        with tc.tile_pool(name="consts", bufs=1) as rpool, \
             tc.tile_pool(name="qkp", bufs=1) as qkpool, \
             tc.tile_pool(name="vp", bufs=1) as vpool, \
             tc.tile_pool(name="ropep", bufs=1) as ropepool:
            ctri_sb = rpool.tile([128, 257], BF16, tag="ctri")
            crow_sb = rpool.tile([1, 128 + GC], BF16, tag="crow")
            sc_sb = rpool.tile([128, 2 * T], BF16, tag="sincos")
            bias_sb = rpool.tile([128, 16], F32, tag="bias")
            tri_sb = ctri_sb[:, 0:128]
            ident_sb = ctri_sb[:, 128:256]
            onescol_sb = ctri_sb[:, 256:257]
            ones128_sb = crow_sb[:, 0:128]
            bv_sb = crow_sb[:, 128:128 + GC]
            sin_sb = sc_sb[:, 0:T]
            cos_sb = sc_sb[:, T:2 * T]

            qk_sb = [qkpool.tile([128, T], BF16, tag=f"qk{m}", name=f"qk{m}")
                     for m in range(16)]
            v_sb = [vpool.tile([128, GC], BF16, tag=f"v{t}", name=f"v{t}")
                    for t in range(TT)]

            def emit_rope(h):
                # in-place RoPE on qk_sb[h] (q) and qk_sb[8+h] (k):
                #   roped = raw * cos2 + swapped(raw) * sin2
                # partition swap via SWDGE (Pool ring) keeps the SP HWDGE
                # ring free for weight/x streaming
                eng = nc.gpsimd if ROPE_DMA == "gpsimd" else nc.sync
                for m in (h, 8 + h):
                    sw = ropepool.tile([128, T], BF16, tag="sw", bufs=1,
                                       name=f"sw{m}")
                    eng.dma_start(out=sw[0:64, :],
                                  in_=qk_sb[m][64:128, :])
                    eng.dma_start(out=sw[64:128, :],
                                  in_=qk_sb[m][0:64, :])
                    tmp = ropepool.tile([128, T], BF16, tag="tmp", bufs=1,
                                        name=f"tmp{m}")
                    nc.vector.tensor_mul(tmp[:], sw[:], sin_sb[:])
                    nc.vector.tensor_mul(sw[:], qk_sb[m][:], cos_sb[:])
                    nc.vector.tensor_add(qk_sb[m][:], tmp[:], sw[:])

            def full_body(iv):
                nc.sync.dma_start(out=ctri_sb[:], in_=ctri)
                nc.sync.dma_start(out=crow_sb[:], in_=crow)
                nc.sync.dma_start(out=sc_sb[:], in_=sincos)
                nc.sync.dma_start(out=bias_sb[:], in_=bqk)

                # ---------- Phases V + QK (x resident, weights streamed) ----
                with tc.tile_pool(name="xp", bufs=1) as xpool, \
                     tc.tile_pool(name="wvp", bufs=1) as wvpool, \
                     tc.tile_pool(name="w1p", bufs=1) as w1pool, \
                     tc.tile_pool(name="ps1", bufs=1, space="PSUM") as ps1:
                    # single x tile: [p, k(16), t(2048)] in two 4MB DMAs
                    xall = xpool.tile([128, KT * T], BF16, tag="xall",
                                      name="xall")
                    step = KT * T // X_SPLIT
                    for s in range(X_SPLIT):
                        nc.sync.dma_start(
                            out=xall[:, s * step:(s + 1) * step],
                            in_=x2[:, s * step:(s + 1) * step])

                    def xsl(k, lo, hi):
                        return xall[:, k * T + lo:k * T + hi]

                    def emit_v_chunk(nd):
                        # v[:, nd*512:(nd+1)*512] = x @ Wv chunk + bias
                        wvc = wvpool.tile([128, KT * 512], BF16, tag="wvc",
                                          bufs=1, name=f"wvc{nd}")
                        nc.sync.dma_start(
                            out=wvc[:],
                            in_=wv[:, nd * KT * 512:(nd + 1) * KT * 512])
                        ndsl = slice(nd * 512, (nd + 1) * 512)
                        for t in range(TT):
                            ps = ps1.tile([128, 512], F32, tag="ps1", bufs=8,
                                          name=f"psv{nd}_{t}")
                            for k in range(KT):
                                nc.tensor.matmul(
                                    ps[:],
                                    xsl(k, t * 128, (t + 1) * 128),
                                    wvc[:, k * 512:(k + 1) * 512],
                                    start=(k == 0), stop=False)
                            nc.tensor.matmul(
                                ps[:], ones128_sb[:], bv_sb[:, ndsl],
                                start=False, stop=True)
                            nc.scalar.copy(v_sb[t][:, ndsl], ps[:])

                    def emit_qk_block(j):
                        # one 512KB DMA per j-block (j even: q_{j//2},
                        # j odd: k_{j//2}); rope after each head's pair.
                        # k-outer/n-inner so each stationary weight tile
                        # serves 4 matmuls (amortizes LDWEIGHTS)
                        m = (j // 2) + 8 * (j % 2)
                        wg = w1pool.tile([128, KT * 128], BF16, tag="wg",
                                         bufs=2, name=f"wg{j}")
                        nc.sync.dma_start(
                            out=wg[:], in_=wqk[:, j * 2048:(j + 1) * 2048])
                        pss = [ps1.tile([128, 512], F32, tag="ps1",
                                        bufs=8, name=f"psqk{m}_{n}")
                               for n in range(TC)]
                        for k in range(KT):
                            for n in range(TC):
                                nc.tensor.matmul(
                                    pss[n][:],
                                    wg[:, k * 128:(k + 1) * 128],
                                    xsl(k, n * 512, (n + 1) * 512),
                                    start=(k == 0), stop=(k == KT - 1))
                        for n in range(TC):
                            nc.vector.tensor_scalar_add(
                                qk_sb[m][:, n * 512:(n + 1) * 512],
                                pss[n][:], bias_sb[:, j:j + 1])
                        if j % 2 == 1:
                            emit_rope(j // 2)

                    emit_v_chunk(0)
                    emit_qk_block(0)
                    emit_qk_block(1)      # hides the wvc reload for nd=1
                    emit_v_chunk(1)
                    for j in range(2, 16):
                        emit_qk_block(j)

                if 2 not in phases and 3 not in phases:
                    return

                # ---------------- Phases 2+3 ----------------
                with tc.tile_pool(name="attnp", bufs=1) as apool, \
                     tc.tile_pool(name="w3p", bufs=1) as w3pool:
                    attn_sb = [
                        apool.tile([128, T], BF16, tag=f"at{h}", name=f"at{h}")
                        for h in range(HG)
                    ]
                    # out-proj weights prefetch during phase 2 in 4 chunks
                    # (ACT HWDGE ring — keeps the SP ring free)
                    wo_sb = w3pool.tile([128, 16 * HG * 128], BF16,
                                        tag="wo", name="wo_sb")
                    for gq in range(4):
                        nc.scalar.dma_start(
                            out=wo_sb[:, gq * 4096:(gq + 1) * 4096],
                            in_=wo[:, gq * 4096:(gq + 1) * 4096])

                    # ---- Phase 2: attention per head ----
                    with tc.tile_pool(name="pexpp", bufs=1) as epool, \
                         tc.tile_pool(name="normp", bufs=1) as npool, \
                         tc.tile_pool(name="ps2s", bufs=2,
                                      space="PSUM") as ps2, \
                         tc.tile_pool(name="ps2o", bufs=2,
                                      space="PSUM") as po2, \
                         tc.tile_pool(name="ps2z", bufs=2,
                                      space="PSUM") as pz2:
                        def make_chunk(h, n):
                            """Emitter for head h, q-chunk n. step(i) emits
                            scores+exp for j-pair i and PV/Z for pair i-1;
                            tail() emits the last PV/Z and the softmax
                            normalization. Two heads' emitters are
                            interleaved step-by-step so the PE always has
                            independent work while ScalarE runs exp."""
                            qr = qk_sb[h]
                            kr = qk_sb[8 + h]
                            jmax = 4 * (n + 1)
                            ps_o = po2.tile([128, 512], F32, tag="po",
                                            name=f"po{h}_{n}")
                            ps_z = pz2.tile([1, 512], F32, tag="pz",
                                            name=f"pz{h}_{n}")
                            acc = npool.tile([128, 512], BF16, tag="zacc",
                                             bufs=2, name=f"zacc{h}_{n}")
                            # j processed descending so diagonal tiles
                            # (width-narrowed) come first and the last
                            # PSUM-group matmul (j=0) is full width.
                            js = list(range(jmax - 1, -1, -1))
                            pairs = [(js[2 * i], js[2 * i + 1])
                                     for i in range(jmax // 2)]

                            def left_of(j):
                                return (j - 4 * n) * 128 \
                                    if (j // 4) == n else 0

                            def emit_pv(pair, pexp):
                                for u, j in enumerate(pair):
                                    lf = left_of(j)
                                    psl = slice(u * 512 + lf, (u + 1) * 512)
                                    nc.tensor.matmul(
                                        ps_o[:, lf:512],
                                        v_sb[j][:, h * 128:(h + 1) * 128],
                                        pexp[:, psl],
                                        start=(j == jmax - 1), stop=(j == 0))

                            state = {}

                            def step(i):
                                pair = pairs[i]
                                ps_s = ps2.tile([128, 1024], F32, tag="ps",
                                                name=f"ps{h}_{n}_{pair[0]}")
                                for u, j in enumerate(pair):
                                    lf = left_of(j)
                                    diag = (j // 4) == n
                                    nc.tensor.matmul(
                                        ps_s[:, u * 512 + lf:(u + 1) * 512],
                                        kr[:, j * 128:(j + 1) * 128],
                                        qr[:, n * 512 + lf:(n + 1) * 512],
                                        start=True, stop=not diag)
                                    if diag:
                                        nc.tensor.matmul(
                                            ps_s[:, u * 512 + lf:
                                                 u * 512 + lf + 128],
                                            ident_sb[:], tri_sb[:],
                                            start=False, stop=True)
                                pexp = epool.tile(
                                    [128, 1024], BF16, tag="pexp", bufs=4,
                                    name=f"pexp{h}_{n}_{pair[0]}")
                                w0 = 512 - left_of(pair[0])
                                w1 = 512 - left_of(pair[1])
                                if w0 + w1 < 672:
                                    # two narrow calls beat one full call
                                    for u, j in enumerate(pair):
                                        lf = left_of(j)
                                        psl = slice(u * 512 + lf,
                                                    (u + 1) * 512)
                                        nc.scalar.activation(
                                            pexp[:, psl], ps_s[:, psl],
                                            mybir.ActivationFunctionType.Exp,
                                            scale=SCALE)
                                else:
                                    nc.scalar.activation(
                                        pexp[:], ps_s[:],
                                        mybir.ActivationFunctionType.Exp,
                                        scale=SCALE)
                                if i == 0:
                                    # defer pair0 acc ops; pair1's full-width
                                    # tile initializes acc with a copy (no
                                    # memset needed)
                                    state["first"] = (pair, pexp)
                                else:
                                    if i == 1:
                                        # pair1 = (4n+1, 4n): lf 128 / 0
                                        nc.vector.tensor_copy(
                                            acc[:], pexp[:, 512:1024])
                                        nc.vector.tensor_add(
                                            acc[:, 128:512], acc[:, 128:512],
                                            pexp[:, 128:512])
                                        fpair, fpexp = state.pop("first")
                                        for u, j in enumerate(fpair):
                                            lf = left_of(j)
                                            psl = slice(u * 512 + lf,
                                                        (u + 1) * 512)
                                            nc.vector.tensor_add(
                                                acc[:, lf:512],
                                                acc[:, lf:512],
                                                fpexp[:, psl])
                                    else:
                                        for u, j in enumerate(pair):
                                            lf = left_of(j)
                                            psl = slice(u * 512 + lf,
                                                        (u + 1) * 512)
                                            nc.vector.tensor_add(
                                                acc[:, lf:512],
                                                acc[:, lf:512],
                                                pexp[:, psl])
                                prev = state.get("prev")
                                if prev is not None:
                                    emit_pv(*prev)
                                state["prev"] = (pair, pexp)

                            def tail():
                                emit_pv(*state["prev"])
                                nc.tensor.matmul(
                                    ps_z[:], onescol_sb[:], acc[:],
                                    start=True, stop=True)
                                qsl = slice(n * 512, (n + 1) * 512)
                                rz = npool.tile([1, 512], BF16, tag="rz",
                                                bufs=2, name=f"rz{h}_{n}")
                                with nc.allow_low_precision(
                                        reason="1/Z applied to bf16 attn"):
                                    nc.vector.reciprocal(rz[:], ps_z[:])
                                # evict unnormalized P@V now so ps_o frees
                                # without waiting on the gpsimd broadcast
                                nc.vector.tensor_scalar_mul(
                                    attn_sb[h][:, qsl], ps_o[:], 1.0)
                                rzb = npool.tile([128, 512], BF16,
                                                 tag="rzb", bufs=2,
                                                 name=f"rzb{h}_{n}")
                                nc.gpsimd.partition_broadcast(rzb[:], rz[:])

                                def mul(h=h, qsl=qsl, rzb=rzb):
                                    nc.vector.tensor_mul(
                                        attn_sb[h][:, qsl],
                                        attn_sb[h][:, qsl], rzb[:])
                                pending_muls.append(mul)

                            return len(pairs), step, tail

                        pending_muls = []
                        for hp in range(HG // 2 if 2 in phases else 0):
                            ha, hb = 2 * hp, 2 * hp + 1
                            for n in range(TC):
                                npairs, step_a, tail_a = make_chunk(ha, n)
                                _, step_b, tail_b = make_chunk(hb, n)
                                for i in range(npairs):
                                    step_a(i)
                                    step_b(i)
                                    if i == min(1, npairs - 1):
                                        # deferred normalize muls from the
                                        # previous chunk: far from both the
                                        # gpsimd broadcast and the evicts
                                        while pending_muls:
                                            pending_muls.pop(0)()
                                tail_a()
                                tail_b()
                        while pending_muls:
                            pending_muls.pop(0)()

                    # ---- Phase 3: out projection (y^T form) ----
                    # stationary = Wo subtile [d, 128 c], moving = attn_sb[h]
                    # t-chunks: one LDWEIGHTS serves 4 matmuls; h accumulates
                    # in PSUM. Output is y^T [c, t]; host transposes.
                    with tc.tile_pool(name="yp", bufs=1) as ypool, \
                         tc.tile_pool(name="ps3", bufs=1,
                                      space="PSUM") as ps3:
                        for g4 in range(8 if 3 in phases else 0):
                            ybufT = ypool.tile([128, 2 * T], BF16,
                                               tag="ybufT", bufs=2,
                                               name=f"ybufT{g4}")
                            for ci in range(2):
                                csub = g4 * 2 + ci
                                pss = [ps3.tile([128, 512], F32, tag="py",
                                                bufs=8,
                                                name=f"py{csub}_{tc_}")
                                       for tc_ in range(4)]
                                for h in range(HG):
                                    for tc_ in range(4):
                                        nc.tensor.matmul(
                                            pss[tc_][:],
                                            wo_sb[:, (csub * HG + h) * 128:
                                                  (csub * HG + h + 1) * 128],
                                            attn_sb[h][:, tc_ * 512:
                                                       (tc_ + 1) * 512],
                                            start=(h == 0),
                                            stop=(h == HG - 1))
                                for tc_ in range(4):
                                    nc.scalar.copy(
                                        ybufT[:, ci * T + tc_ * 512:
                                              ci * T + (tc_ + 1) * 512],
                                        pss[tc_][:])
                            nc.sync.dma_start(
                                out=y[:, g4 * 2:(g4 + 1) * 2, :],
                                in_=ybufT[:])

            if unroll > 1:
                for _ in range(unroll):
                    full_body(None)
            elif iters == 1:
                full_body(None)
            else:
                with tc.For_i(0, iters, 1) as iv:
                    full_body(iv)

    nc.compile()
    return nc


def make_host_inputs(x, Wqkv, bqkv, Wo):
    """Per-core input maps (host-side sharding + bf16 conversion)."""
    half = D // 2
    freq = np.arange(half, dtype=np.float64)
    theta = 1.0 / (10000.0 ** (2.0 * freq / D))
    pos = np.arange(T, dtype=np.float64)
    ang = pos[:, None] * theta[None, :]          # [T, half]
    sinT = np.sin(ang).T.astype(np.float32)      # [half, T]
    cosT = np.cos(ang).T.astype(np.float32)
    # sign folded into the sin table for the partition-swap RoPE form
    sin2 = np.concatenate([-sinT, sinT], axis=0).astype(np.float32)
    cos2 = np.concatenate([cosT, cosT], axis=0).astype(np.float32)
    sincos = np.concatenate([sin2, cos2], axis=1).astype(BF_NP)  # [128, 2T]

    f = np.arange(128)[None, :]
    p = np.arange(128)[:, None]
    tri = np.where(f >= p, 0.0, MASKVAL).astype(np.float32)
    ident = np.eye(128, dtype=np.float32)
    onescol = np.ones((128, 1), dtype=np.float32)
    ctri = np.concatenate([tri, ident, onescol], axis=1).astype(BF_NP)

    ones128 = np.ones((1, 128), dtype=np.float32)

    # interleaved block order j -> m (q0,k0,q1,k1,...)
    perm = np.array([(j // 2) + 8 * (j % 2) for j in range(16)])

    # x pre-tiled [p, k, t]
    x2 = [np.ascontiguousarray(
        x[b].T.reshape(KT, 128, T).transpose(1, 0, 2).reshape(128, KT * T)
    ).astype(BF_NP) for b in range(B)]

    in_maps = []
    for core in range(N_CORES):
        b, g = core // 2, core % 2
        cs = slice(g * GC, (g + 1) * GC)
        Wq = Wqkv[:, :C][:, cs]
        Wk = Wqkv[:, C:2 * C][:, cs]
        Wv = Wqkv[:, 2 * C:][:, cs]
        # [C, 2*GC] -> blocks [c, m, d] -> j-order -> [p, j, k, d]
        Wqk = np.concatenate([Wq, Wk], axis=1).reshape(C, 16, 128)
        Wqk = Wqk[:, perm, :].reshape(C, 2048)
        wqk_r = np.ascontiguousarray(
            Wqk.reshape(KT, 128, 16, 128).transpose(1, 2, 0, 3)
            .reshape(128, 16 * KT * 128)).astype(BF_NP)
        # bias [m(16), d(128)] -> j-order -> [d(p), j]
        bqk_r = np.concatenate([bqkv[:C][cs], bqkv[C:2 * C][cs]])
        bqk_r = np.ascontiguousarray(
            bqk_r.reshape(16, 128)[perm].T).astype(np.float32)
        # [C, GC] -> [p, nd, k, c] -> [128, 2*16*512]
        wv_r = np.ascontiguousarray(
            Wv.reshape(KT, 128, ND, 512).transpose(1, 2, 0, 3)
            .reshape(128, ND * KT * 512)).astype(BF_NP)
        bv_r = bqkv[2 * C:][cs].reshape(1, GC).astype(np.float32)
        crow = np.ascontiguousarray(
            np.concatenate([ones128, bv_r], axis=1)).astype(BF_NP)
        # [GC, C] -> [h, d, csub, c] -> [d, csub, h, c] -> [128, 16*8*128]
        wo_r = np.ascontiguousarray(
            Wo[cs, :].reshape(HG, 128, 16, 128).transpose(1, 2, 0, 3)
            .reshape(128, 16 * HG * 128)).astype(BF_NP)
        in_maps.append({
            "x2": x2[b],
            "wqk": wqk_r,
            "bqk": bqk_r,
            "wv": wv_r,
            "wo": wo_r,
            "sincos": sincos,
            "ctri": ctri,
            "crow": crow,
        })
    return in_maps


_PROGRAM_CACHE = {}


def get_program(iters=1):
    if iters not in _PROGRAM_CACHE:
        _PROGRAM_CACHE[iters] = build_program(iters)
    return _PROGRAM_CACHE[iters]


def kernel(x, Wqkv, bqkv, Wo, bo):
    x = np.asarray(x, dtype=np.float32)
    Wqkv = np.asarray(Wqkv, dtype=np.float32)
    bqkv = np.asarray(bqkv, dtype=np.float32)
    Wo = np.asarray(Wo, dtype=np.float32)
    bo = np.asarray(bo, dtype=np.float32)

    nc = get_program(1)
    in_maps = make_host_inputs(x, Wqkv, bqkv, Wo)
    res = run_bass_kernel_spmd(nc, in_maps, list(range(N_CORES)))

    def unpack(yr):
        # y^T [p, csub, t] -> y [t, C=(csub,p)]
        yr = np.asarray(yr, dtype=np.float32)
        return yr.transpose(2, 1, 0).reshape(T, C)

    out = np.empty((B, T, C), dtype=np.float32)
    for b in range(B):
        out[b] = (unpack(res.results[2 * b]["y"])
                  + unpack(res.results[2 * b + 1]["y"]) + bo)
    return out



# revision 51
# speedup vs baseline: 1.0584x; 1.0584x over previous
"""nn_MultiHeadAttention kernel for 8 Trainium2 NeuronCores.

Sharding: 8 cores = 4 batches (data parallel) x 2 head-groups of 8 heads
(tensor parallel). Each core computes its batch's QKV projection for its
head group (column-parallel), RoPE, causal attention, and a partial
out-projection (row-parallel). Host sums the two partials per batch and
adds the output bias.

Design (vs the DRAM-spill f32r baseline):
  - All matmul operands bf16 (host pre-converts); fp32 PSUM accumulation.
  - Everything SBUF-resident: x (8MB), q/k rows (8MB, roped in place),
    v (4MB), attn (4MB). No DRAM scratch roundtrips.
  - Weights host-prearranged to [128, ...] layouts so every weight DMA is
    a contiguous slice (4KB+/partition lines). Consts packed into 3 DMAs;
    x pre-tiled on host to [p, k, t] (2x 4MB DMAs); QK bias as one
    [128, 16] tensor. RoPE's partition-swap SBUF-SBUF copies issue on the
    GPSIMD SWDGE ring so the SP HWDGE ring stays free for weight/x
    streaming (measured ~38us/iter on HW).
  - QK proj k-outer/n-inner: each stationary weight tile feeds 4
    consecutive matmuls into 4 PSUM banks (bufs=8 ring).
  - Attention processes two heads' chunks interleaved step-by-step so the
    PE always has independent work while ScalarE runs exp, with score
    matmuls one j-pair ahead of the PV matmuls. Causal structure:
    j-tiles in descending order with diagonal tiles width-narrowed; only
    the 128x128 diagonal block is masked, by accumulating a shared
    triangular table through a 128-row identity matmul.
  - Scores stay transposed S^T[k,q]; exp on ScalarE straight from PSUM
    with the 1/sqrt(D) scale fused. The softmax denominator accumulates
    on the DVE (SBUF bf16 tile, one add per pexp tile) with a single
    ones-column matmul per chunk — keeping PE work per pipeline cycle
    well below the ScalarE exp pace so HAM clock-gate oscillation cannot
    throttle the PE (this alone was worth ~73us/iter).
  - Normalization is decoupled from the PSUM critical path: unnormalized
    P@V evicts immediately (DVE); the 1/Z multiply (DVE reciprocal +
    GPSIMD partition-broadcast) is deferred one chunk.
  - Out-projection computes y^T: stationary = Wo [d, 128c] subtiles (all
    of Wo prefetched to SBUF during attention on the ACT HWDGE ring),
    moving = attn rows, h accumulated in PSUM; one stationary serves 4
    t-chunk matmuls. y lands as bf16 [p, csub, t]; the host transposes
    and sums the two head-group partials in fp32 (measured ~48us/iter
    over the m-major form with per-tile y DMAs).

HW notes (measured on trn2 via For_i-loop deltas): a dependency-free
stream of N=512 bf16 matmuls with per-MM stationary reload runs at
~286 ns/MM (not the 213 ns theoretical), which puts this kernel's PE
floor around 820us/iter. Explicitly dropping redundant LDWEIGHTS via
InstMatmult.ldweights=False measured SLOWER (~+24 ns/MM), as did fp8;
DMA count consolidation was neutral apart from the SWDGE rope offload.
"""

import os
import sys

if "/opt/trn_rl_repo" not in sys.path:
    sys.path.insert(0, "/opt/trn_rl_repo")

ROPE_DMA = os.environ.get("K_ROPE_DMA", "gpsimd")   # sync | gpsimd
X_SPLIT = int(os.environ.get("K_X_SPLIT", "2"))     # number of x DMAs

import numpy as np
import ml_dtypes

import concourse.bass as bass
import concourse.bacc as bacc
import concourse.mybir as mybir
import concourse.tile as tile
from concourse.bass_utils import run_bass_kernel_spmd

F32 = mybir.dt.float32
BF16 = mybir.dt.bfloat16
BF_NP = ml_dtypes.bfloat16

B, T, C = 4, 2048, 2048
H = 16            # total heads
HG = 8            # heads per core (group)
D = 128           # head dim
GC = HG * D       # channels per group = 1024
SCALE = 1.0 / float(np.sqrt(D))
MASKVAL = -30000.0
N_CORES = 8

KT = C // 128     # 16 contraction tiles
TT = T // 128     # 16 T tiles
TC = T // 512     # 4 T chunks of 512
ND = 2            # v output chunks of 512


def build_program(iters=1, unroll=1, phases=(1, 2, 3)):
    nc = bacc.Bacc("TRN2", target_bir_lowering=False, debug=False)

    # x pre-tiled: [p=128, k(16), t(2048)] — block k holds x^T rows k*128..
    x2 = nc.dram_tensor("x2", [128, KT * T], BF16, kind="ExternalInput").ap()
    # j-major (j = q0,k0,q1,k1,...): [p=128, j(16), k(16), c(128)]
    wqk = nc.dram_tensor("wqk", [128, 16 * KT * 128], BF16,
                         kind="ExternalInput").ap()
    # [p=128 (out dim within block), j(16)]
    bqk = nc.dram_tensor("bqk", [128, 16], F32, kind="ExternalInput").ap()
    # nd-major: [p=128, nd(2), k(16), c(512)]
    wv = nc.dram_tensor("wv", [128, ND * KT * 512], BF16,
                        kind="ExternalInput").ap()
    # csub-major: [p=128 (d), csub(16), h(8), c(128)]
    wo = nc.dram_tensor("wo", [128, 16 * HG * 128], BF16,
                        kind="ExternalInput").ap()
    # packed consts: sincos = [sin2 | cos2], ctri = [tri | ident | onescol],
    # crow = [ones128 | bv]
    sincos = nc.dram_tensor("sincos", [128, 2 * T], BF16,
                            kind="ExternalInput").ap()
    ctri = nc.dram_tensor("ctri", [128, 257], BF16,
                          kind="ExternalInput").ap()
    crow = nc.dram_tensor("crow", [1, 128 + GC], BF16,
                          kind="ExternalInput").ap()
    # y^T stored [p(128), csub(16), t(2048)]; host reassembles
    y = nc.dram_tensor("y", [128, 16, T], BF16,
                       kind="ExternalOutput").ap()

    with tile.TileContext(nc) as tc:
        with tc.tile_pool(name="consts", bufs=1) as rpool, \
             tc.tile_pool(name="qkp", bufs=1) as qkpool, \
             tc.tile_pool(name="vp", bufs=1) as vpool, \
             tc.tile_pool(name="ropep", bufs=1) as ropepool:
            ctri_sb = rpool.tile([128, 257], BF16, tag="ctri")
            crow_sb = rpool.tile([1, 128 + GC], BF16, tag="crow")
            sc_sb = rpool.tile([128, 2 * T], BF16, tag="sincos")
            bias_sb = rpool.tile([128, 16], F32, tag="bias")
            tri_sb = ctri_sb[:, 0:128]
            ident_sb = ctri_sb[:, 128:256]
            onescol_sb = ctri_sb[:, 256:257]
            ones128_sb = crow_sb[:, 0:128]
            bv_sb = crow_sb[:, 128:128 + GC]
            sin_sb = sc_sb[:, 0:T]
            cos_sb = sc_sb[:, T:2 * T]

            qk_sb = [qkpool.tile([128, T], BF16, tag=f"qk{m}", name=f"qk{m}")
                     for m in range(16)]
            v_sb = [vpool.tile([128, GC], BF16, tag=f"v{t}", name=f"v{t}")
                    for t in range(TT)]

            def emit_rope(h):
                # in-place RoPE on qk_sb[h] (q) and qk_sb[8+h] (k):
                #   roped = raw * cos2 + swapped(raw) * sin2
                # partition swap via SWDGE (Pool ring) keeps the SP HWDGE
                # ring free for weight/x streaming
                eng = nc.gpsimd if ROPE_DMA == "gpsimd" else nc.sync
                for m in (h, 8 + h):
                    sw = ropepool.tile([128, T], BF16, tag="sw", bufs=1,
                                       name=f"sw{m}")
                    eng.dma_start(out=sw[0:64, :],
                                  in_=qk_sb[m][64:128, :])
                    eng.dma_start(out=sw[64:128, :],
                                  in_=qk_sb[m][0:64, :])
                    tmp = ropepool.tile([128, T], BF16, tag="tmp", bufs=1,
                                        name=f"tmp{m}")
                    nc.vector.tensor_mul(tmp[:], sw[:], sin_sb[:])
                    nc.vector.tensor_mul(sw[:], qk_sb[m][:], cos_sb[:])
                    nc.vector.tensor_add(qk_sb[m][:], tmp[:], sw[:])

            def full_body(iv):
                nc.sync.dma_start(out=ctri_sb[:], in_=ctri)
                nc.sync.dma_start(out=crow_sb[:], in_=crow)
                nc.sync.dma_start(out=sc_sb[:], in_=sincos)
                nc.sync.dma_start(out=bias_sb[:], in_=bqk)

                # ---------- Phases V + QK (x resident, weights streamed) ----
                with tc.tile_pool(name="xp", bufs=1) as xpool, \
                     tc.tile_pool(name="wvp", bufs=1) as wvpool, \
                     tc.tile_pool(name="w1p", bufs=1) as w1pool, \
                     tc.tile_pool(name="ps1", bufs=1, space="PSUM") as ps1:
                    # single x tile: [p, k(16), t(2048)] in two 4MB DMAs
                    xall = xpool.tile([128, KT * T], BF16, tag="xall",
                                      name="xall")
                    step = KT * T // X_SPLIT
                    for s in range(X_SPLIT):
                        nc.sync.dma_start(
                            out=xall[:, s * step:(s + 1) * step],
                            in_=x2[:, s * step:(s + 1) * step])

                    def xsl(k, lo, hi):
                        return xall[:, k * T + lo:k * T + hi]

                    def emit_v_chunk(nd):
                        # v[:, nd*512:(nd+1)*512] = x @ Wv chunk + bias
                        wvc = wvpool.tile([128, KT * 512], BF16, tag="wvc",
                                          bufs=1, name=f"wvc{nd}")
                        nc.sync.dma_start(
                            out=wvc[:],
                            in_=wv[:, nd * KT * 512:(nd + 1) * KT * 512])
                        ndsl = slice(nd * 512, (nd + 1) * 512)
                        for t in range(TT):
                            ps = ps1.tile([128, 512], F32, tag="ps1", bufs=8,
                                          name=f"psv{nd}_{t}")
                            for k in range(KT):
                                nc.tensor.matmul(
                                    ps[:],
                                    xsl(k, t * 128, (t + 1) * 128),
                                    wvc[:, k * 512:(k + 1) * 512],
                                    start=(k == 0), stop=False)
                            nc.tensor.matmul(
                                ps[:], ones128_sb[:], bv_sb[:, ndsl],
                                start=False, stop=True)
                            nc.scalar.copy(v_sb[t][:, ndsl], ps[:])

                    def emit_qk_block(j):
                        # one 512KB DMA per j-block (j even: q_{j//2},
                        # j odd: k_{j//2}); rope after each head's pair.
                        # k-outer/n-inner so each stationary weight tile
                        # serves 4 matmuls (amortizes LDWEIGHTS)
                        m = (j // 2) + 8 * (j % 2)
                        wg = w1pool.tile([128, KT * 128], BF16, tag="wg",
                                         bufs=2, name=f"wg{j}")
                        nc.sync.dma_start(
                            out=wg[:], in_=wqk[:, j * 2048:(j + 1) * 2048])
                        pss = [ps1.tile([128, 512], F32, tag="ps1",
                                        bufs=8, name=f"psqk{m}_{n}")
                               for n in range(TC)]
                        for k in range(KT):
                            for n in range(TC):
                                nc.tensor.matmul(
                                    pss[n][:],
                                    wg[:, k * 128:(k + 1) * 128],
                                    xsl(k, n * 512, (n + 1) * 512),
                                    start=(k == 0), stop=(k == KT - 1))
                        for n in range(TC):
                            nc.vector.tensor_scalar_add(
                                qk_sb[m][:, n * 512:(n + 1) * 512],
                                pss[n][:], bias_sb[:, j:j + 1])
                        if j % 2 == 1:
                            emit_rope(j // 2)

                    emit_v_chunk(0)
                    emit_qk_block(0)
                    emit_qk_block(1)      # hides the wvc reload for nd=1
                    emit_v_chunk(1)
                    for j in range(2, 16):
                        emit_qk_block(j)

                if 2 not in phases and 3 not in phases:
                    return

                # ---------------- Phases 2+3 ----------------
                with tc.tile_pool(name="attnp", bufs=1) as apool, \
                     tc.tile_pool(name="w3p", bufs=1) as w3pool:
                    attn_sb = [
                        apool.tile([128, T], BF16, tag=f"at{h}", name=f"at{h}")
                        for h in range(HG)
                    ]
                    # out-proj weights prefetch during phase 2 in 4 chunks
                    # (ACT HWDGE ring — keeps the SP ring free)
                    wo_sb = w3pool.tile([128, 16 * HG * 128], BF16,
                                        tag="wo", name="wo_sb")
                    for gq in range(4):
                        nc.scalar.dma_start(
                            out=wo_sb[:, gq * 4096:(gq + 1) * 4096],
                            in_=wo[:, gq * 4096:(gq + 1) * 4096])

                    # ---- Phase 2: attention per head ----
                    with tc.tile_pool(name="pexpp", bufs=1) as epool, \
                         tc.tile_pool(name="normp", bufs=1) as npool, \
                         tc.tile_pool(name="ps2s", bufs=2,
                                      space="PSUM") as ps2, \
                         tc.tile_pool(name="ps2o", bufs=2,
                                      space="PSUM") as po2, \
                         tc.tile_pool(name="ps2z", bufs=2,
                                      space="PSUM") as pz2:
                        def make_chunk(h, n):
                            """Emitter for head h, q-chunk n. step(i) emits
                            scores+exp for j-pair i and PV/Z for pair i-1;
                            tail() emits the last PV/Z and the softmax
                            normalization. Two heads' emitters are
                            interleaved step-by-step so the PE always has
                            independent work while ScalarE runs exp."""
                            qr = qk_sb[h]
                            kr = qk_sb[8 + h]
                            jmax = 4 * (n + 1)
                            ps_o = po2.tile([128, 512], F32, tag="po",
                                            name=f"po{h}_{n}")
                            ps_z = pz2.tile([1, 512], F32, tag="pz",
                                            name=f"pz{h}_{n}")
                            acc = npool.tile([128, 512], BF16, tag="zacc",
                                             bufs=2, name=f"zacc{h}_{n}")
                            nc.vector.memset(acc[:], 0.0)
                            # j processed descending so diagonal tiles
                            # (width-narrowed) come first and the last
                            # PSUM-group matmul (j=0) is full width.
                            js = list(range(jmax - 1, -1, -1))
                            pairs = [(js[2 * i], js[2 * i + 1])
                                     for i in range(jmax // 2)]

                            def left_of(j):
                                return (j - 4 * n) * 128 \
                                    if (j // 4) == n else 0

                            def emit_pv(pair, pexp):
                                for u, j in enumerate(pair):
                                    lf = left_of(j)
                                    psl = slice(u * 512 + lf, (u + 1) * 512)
                                    nc.tensor.matmul(
                                        ps_o[:, lf:512],
                                        v_sb[j][:, h * 128:(h + 1) * 128],
                                        pexp[:, psl],
                                        start=(j == jmax - 1), stop=(j == 0))

                            state = {}

                            def step(i):
                                pair = pairs[i]
                                ps_s = ps2.tile([128, 1024], F32, tag="ps",
                                                name=f"ps{h}_{n}_{pair[0]}")
                                for u, j in enumerate(pair):
                                    lf = left_of(j)
                                    diag = (j // 4) == n
                                    nc.tensor.matmul(
                                        ps_s[:, u * 512 + lf:(u + 1) * 512],
                                        kr[:, j * 128:(j + 1) * 128],
                                        qr[:, n * 512 + lf:(n + 1) * 512],
                                        start=True, stop=not diag)
                                    if diag:
                                        nc.tensor.matmul(
                                            ps_s[:, u * 512 + lf:
                                                 u * 512 + lf + 128],
                                            ident_sb[:], tri_sb[:],
                                            start=False, stop=True)
                                pexp = epool.tile(
                                    [128, 1024], BF16, tag="pexp", bufs=4,
                                    name=f"pexp{h}_{n}_{pair[0]}")
                                w0 = 512 - left_of(pair[0])
                                w1 = 512 - left_of(pair[1])
                                if w0 + w1 < 672:
                                    # two narrow calls beat one full call
                                    for u, j in enumerate(pair):
                                        lf = left_of(j)
                                        psl = slice(u * 512 + lf,
                                                    (u + 1) * 512)
                                        nc.scalar.activation(
                                            pexp[:, psl], ps_s[:, psl],
                                            mybir.ActivationFunctionType.Exp,
                                            scale=SCALE)
                                else:
                                    nc.scalar.activation(
                                        pexp[:], ps_s[:],
                                        mybir.ActivationFunctionType.Exp,
                                        scale=SCALE)
                                for u, j in enumerate(pair):
                                    lf = left_of(j)
                                    psl = slice(u * 512 + lf, (u + 1) * 512)
                                    nc.vector.tensor_add(
                                        acc[:, lf:512], acc[:, lf:512],
                                        pexp[:, psl])
                                prev = state.get("prev")
                                if prev is not None:
                                    emit_pv(*prev)
                                state["prev"] = (pair, pexp)

                            def tail():
                                emit_pv(*state["prev"])
                                nc.tensor.matmul(
                                    ps_z[:], onescol_sb[:], acc[:],
                                    start=True, stop=True)
                                qsl = slice(n * 512, (n + 1) * 512)
                                rz = npool.tile([1, 512], F32, tag="rz",
                                                bufs=2, name=f"rz{h}_{n}")
                                nc.vector.reciprocal(rz[:], ps_z[:])
                                # evict unnormalized P@V now so ps_o frees
                                # without waiting on the gpsimd broadcast
                                nc.vector.tensor_scalar_mul(
                                    attn_sb[h][:, qsl], ps_o[:], 1.0)
                                rzb = npool.tile([128, 512], F32, tag="rzb",
                                                 bufs=2, name=f"rzb{h}_{n}")
                                nc.gpsimd.partition_broadcast(rzb[:], rz[:])

                                def mul(h=h, qsl=qsl, rzb=rzb):
                                    nc.vector.tensor_mul(
                                        attn_sb[h][:, qsl],
                                        attn_sb[h][:, qsl], rzb[:])
                                pending_muls.append(mul)

                            return len(pairs), step, tail

                        pending_muls = []
                        for hp in range(HG // 2 if 2 in phases else 0):
                            ha, hb = 2 * hp, 2 * hp + 1
                            for n in range(TC):
                                npairs, step_a, tail_a = make_chunk(ha, n)
                                _, step_b, tail_b = make_chunk(hb, n)
                                for i in range(npairs):
                                    step_a(i)
                                    step_b(i)
                                    if i == 0:
                                        # deferred normalize muls from the
                                        # previous chunk: far from both the
                                        # gpsimd broadcast and the evicts
                                        while pending_muls:
                                            pending_muls.pop(0)()
                                tail_a()
                                tail_b()
                        while pending_muls:
                            pending_muls.pop(0)()

                    # ---- Phase 3: out projection (y^T form) ----
                    # stationary = Wo subtile [d, 128 c], moving = attn_sb[h]
                    # t-chunks: one LDWEIGHTS serves 4 matmuls; h accumulates
                    # in PSUM. Output is y^T [c, t]; host transposes.
                    with tc.tile_pool(name="yp", bufs=1) as ypool, \
                         tc.tile_pool(name="ps3", bufs=1,
                                      space="PSUM") as ps3:
                        for g4 in range(8 if 3 in phases else 0):
                            ybufT = ypool.tile([128, 2 * T], BF16,
                                               tag="ybufT", bufs=2,
                                               name=f"ybufT{g4}")
                            for ci in range(2):
                                csub = g4 * 2 + ci
                                pss = [ps3.tile([128, 512], F32, tag="py",
                                                bufs=8,
                                                name=f"py{csub}_{tc_}")
                                       for tc_ in range(4)]
                                for h in range(HG):
                                    for tc_ in range(4):
                                        nc.tensor.matmul(
                                            pss[tc_][:],
                                            wo_sb[:, (csub * HG + h) * 128:
                                                  (csub * HG + h + 1) * 128],
                                            attn_sb[h][:, tc_ * 512:
                                                       (tc_ + 1) * 512],
                                            start=(h == 0),
                                            stop=(h == HG - 1))
                                for tc_ in range(4):
                                    nc.scalar.copy(
                                        ybufT[:, ci * T + tc_ * 512:
                                              ci * T + (tc_ + 1) * 512],
                                        pss[tc_][:])
                            nc.sync.dma_start(
                                out=y[:, g4 * 2:(g4 + 1) * 2, :],
                                in_=ybufT[:])

            if unroll > 1:
                for _ in range(unroll):
                    full_body(None)
            elif iters == 1:
                full_body(None)
            else:
                with tc.For_i(0, iters, 1) as iv:
                    full_body(iv)

    nc.compile()
    return nc


def make_host_inputs(x, Wqkv, bqkv, Wo):
    """Per-core input maps (host-side sharding + bf16 conversion)."""
    half = D // 2
    freq = np.arange(half, dtype=np.float64)
    theta = 1.0 / (10000.0 ** (2.0 * freq / D))
    pos = np.arange(T, dtype=np.float64)
    ang = pos[:, None] * theta[None, :]          # [T, half]
    sinT = np.sin(ang).T.astype(np.float32)      # [half, T]
    cosT = np.cos(ang).T.astype(np.float32)
    # sign folded into the sin table for the partition-swap RoPE form
    sin2 = np.concatenate([-sinT, sinT], axis=0).astype(np.float32)
    cos2 = np.concatenate([cosT, cosT], axis=0).astype(np.float32)
    sincos = np.concatenate([sin2, cos2], axis=1).astype(BF_NP)  # [128, 2T]

    f = np.arange(128)[None, :]
    p = np.arange(128)[:, None]
    tri = np.where(f >= p, 0.0, MASKVAL).astype(np.float32)
    ident = np.eye(128, dtype=np.float32)
    onescol = np.ones((128, 1), dtype=np.float32)
    ctri = np.concatenate([tri, ident, onescol], axis=1).astype(BF_NP)

    ones128 = np.ones((1, 128), dtype=np.float32)

    # interleaved block order j -> m (q0,k0,q1,k1,...)
    perm = np.array([(j // 2) + 8 * (j % 2) for j in range(16)])

    # x pre-tiled [p, k, t]
    x2 = [np.ascontiguousarray(
        x[b].T.reshape(KT, 128, T).transpose(1, 0, 2).reshape(128, KT * T)
    ).astype(BF_NP) for b in range(B)]

    in_maps = []
    for core in range(N_CORES):
        b, g = core // 2, core % 2
        cs = slice(g * GC, (g + 1) * GC)
        Wq = Wqkv[:, :C][:, cs]
        Wk = Wqkv[:, C:2 * C][:, cs]
        Wv = Wqkv[:, 2 * C:][:, cs]
        # [C, 2*GC] -> blocks [c, m, d] -> j-order -> [p, j, k, d]
        Wqk = np.concatenate([Wq, Wk], axis=1).reshape(C, 16, 128)
        Wqk = Wqk[:, perm, :].reshape(C, 2048)
        wqk_r = np.ascontiguousarray(
            Wqk.reshape(KT, 128, 16, 128).transpose(1, 2, 0, 3)
            .reshape(128, 16 * KT * 128)).astype(BF_NP)
        # bias [m(16), d(128)] -> j-order -> [d(p), j]
        bqk_r = np.concatenate([bqkv[:C][cs], bqkv[C:2 * C][cs]])
        bqk_r = np.ascontiguousarray(
            bqk_r.reshape(16, 128)[perm].T).astype(np.float32)
        # [C, GC] -> [p, nd, k, c] -> [128, 2*16*512]
        wv_r = np.ascontiguousarray(
            Wv.reshape(KT, 128, ND, 512).transpose(1, 2, 0, 3)
            .reshape(128, ND * KT * 512)).astype(BF_NP)
        bv_r = bqkv[2 * C:][cs].reshape(1, GC).astype(np.float32)
        crow = np.ascontiguousarray(
            np.concatenate([ones128, bv_r], axis=1)).astype(BF_NP)
        # [GC, C] -> [h, d, csub, c] -> [d, csub, h, c] -> [128, 16*8*128]
        wo_r = np.ascontiguousarray(
            Wo[cs, :].reshape(HG, 128, 16, 128).transpose(1, 2, 0, 3)
            .reshape(128, 16 * HG * 128)).astype(BF_NP)
        in_maps.append({
            "x2": x2[b],
            "wqk": wqk_r,
            "bqk": bqk_r,
            "wv": wv_r,
            "wo": wo_r,
            "sincos": sincos,
            "ctri": ctri,
            "crow": crow,
        })
    return in_maps


_PROGRAM_CACHE = {}


def get_program(iters=1):
    if iters not in _PROGRAM_CACHE:
        _PROGRAM_CACHE[iters] = build_program(iters)
    return _PROGRAM_CACHE[iters]


def kernel(x, Wqkv, bqkv, Wo, bo):
    x = np.asarray(x, dtype=np.float32)
    Wqkv = np.asarray(Wqkv, dtype=np.float32)
    bqkv = np.asarray(bqkv, dtype=np.float32)
    Wo = np.asarray(Wo, dtype=np.float32)
    bo = np.asarray(bo, dtype=np.float32)

    nc = get_program(1)
    in_maps = make_host_inputs(x, Wqkv, bqkv, Wo)
    res = run_bass_kernel_spmd(nc, in_maps, list(range(N_CORES)))

    def unpack(yr):
        # y^T [p, csub, t] -> y [t, C=(csub,p)]
        yr = np.asarray(yr, dtype=np.float32)
        return yr.transpose(2, 1, 0).reshape(T, C)

    out = np.empty((B, T, C), dtype=np.float32)
    for b in range(B):
        out[b] = (unpack(res.results[2 * b]["y"])
                  + unpack(res.results[2 * b + 1]["y"]) + bo)
    return out

